# revision 10
# baseline (speedup 1.0000x reference)
"""Trainium2 Bass kernel for BatteryMoEFlattenIntraCycleMoELayer.

Computation (reference):
    gates = renorm(top2(softmax(logits) * mask))          # [B, E]
    x = cycle_curve_data.reshape(B, L, 900)
    out[b] = sum_e gates[b,e] * (x[b] @ W[e] + b[e])      # -> bf16 [B, L, 512]

Strategy:
  - Host: compute gates + top-2 routing (tiny), transpose x to feat-major
    [B, 901, 128] with a constant-1.0 row appended (folds the bias add into
    the matmul via weight augmentation W_aug = [W; b]).
  - Shard B across 8 cores (64 samples each). One SPMD program: routing is
    carried as *data* (per-sample W-slot element offsets, read into PE
    registers at runtime -> dynamic access patterns on the matmul moving
    operand), so the program does not depend on input values.
  - Device per sample: 2 experts x 8 K-chunks matmuls (N=512, float32r at
    full PE rate) accumulate x_aug @ W_aug[e] into 2 PSUM banks; ACT engine
    scales each by its gate (per-partition scalar AP from data); DVE adds
    and casts to bf16.
"""

import os
import sys

for _p in ("/opt/trn_rl_repo", "/root/.axon_site/_ro/trn_rl_repo"):
    if os.path.isdir(_p) and _p not in sys.path:
        sys.path.insert(0, _p)

import numpy as np
import ml_dtypes

import concourse.bass as bass
import concourse.mybir as mybir
import concourse.tile as tile
from concourse import bacc
from concourse.bass_utils import run_bass_kernel_spmd
from concourse.bass_values import RuntimeValue

B, L, CURVE_LEN = 512, 128, 300
FEAT = 3 * CURVE_LEN          # 900
FEAT_AUG = FEAT + 1           # 901 (bias row)
D_MODEL = 512
NUM_EXPERTS = 8
TOP_K = 2
EPS = 1e-9
N_CORES = 8
S = B // N_CORES              # 64 samples per core
N_KCH = 8                     # K chunks: 7 x 128 + 1 x 5
K_LAST = FEAT_AUG - 7 * 128   # 5

# matmul input dtype: float32r streams fp32 bits at full PE rate (N>=256)
MM_DT = mybir.dt.float32r

_CACHE = {}


def _kchunk(k):
    return 128 if k < 7 else K_LAST


def _build_nc():
    """Build the SPMD Bass program (routing-independent)."""
    nc = bacc.Bacc(trn_type="TRN2")
    f32 = mybir.dt.float32
    bf16 = mybir.dt.bfloat16
    i32 = mybir.dt.int32

    xt_h = nc.declare_dram_parameter("xt", [S, FEAT_AUG, L], MM_DT, isOutput=False)
    # w laid out per k-chunk: [k, part(<=128), expert, 512]
    w_h = nc.declare_dram_parameter("w", [N_KCH, 128, NUM_EXPERTS, D_MODEL], MM_DT,
                                    isOutput=False)
    g_h = nc.declare_dram_parameter("g", [128, 2 * S], f32, isOutput=False)
    widx_h = nc.declare_dram_parameter("widx", [1, 2 * S], i32, isOutput=False)
    y_h = nc.declare_dram_parameter("y", [S, L, D_MODEL], bf16, isOutput=True)

    with tile.TileContext(nc) as tc:
        with (
            tc.tile_pool(name="wpool", bufs=1) as wpool,
            tc.tile_pool(name="cpool", bufs=1) as cpool,
            tc.tile_pool(name="xpool", bufs=3) as xpool,
            tc.tile_pool(name="tpool", bufs=4) as tpool,
            tc.tile_pool(name="opool", bufs=3) as opool,
            tc.tile_pool(name="pspool", bufs=4, space="PSUM") as pspool,
        ):
            # --- constants: gates, routing offsets, weights ---
            g_sb = cpool.tile([128, 2 * S], f32)
            nc.sync.dma_start(out=g_sb[:, :], in_=g_h[:, :])
            widx_sb = cpool.tile([1, 2 * S], i32)
            nc.sync.dma_start(out=widx_sb[:, :], in_=widx_h[:, :])

            w_sb = []
            for k in range(N_KCH):
                kk = _kchunk(k)
                wt = cpool.tile([128, NUM_EXPERTS * D_MODEL], MM_DT,
                                name=f"w_sb_{k}")
                nc.sync.dma_start(
                    out=wt[:kk, :],
                    in_=w_h[k, :kk, :, :].rearrange("p e d -> p (e d)"),
                )
                w_sb.append(wt)

            # ring of PE registers for the per-sample W-slot offsets
            NRING = 16
            wregs = [nc.tensor.alloc_register(f"widx_reg{i}") for i in range(NRING)]
            WMAX = (NUM_EXPERTS - 1) * D_MODEL

            # --- main loop over samples ---
            for s in range(S):
                x_sb = xpool.tile([128, N_KCH * 128], MM_DT, tag="x")
                # chunks 0..6 in one DMA: partition p <- feat row k*128+p
                nc.sync.dma_start(
                    out=x_sb[:, : 7 * 128].rearrange("p (k l) -> p k l", k=7),
                    in_=xt_h[s, : 7 * 128, :].rearrange("(k p) l -> p k l", p=128),
                )
                # chunk 7: rows 896..900 (4 feat + bias row)
                nc.sync.dma_start(
                    out=x_sb[:K_LAST, 7 * 128: 8 * 128],
                    in_=xt_h[s, 7 * 128: FEAT_AUG, :],
                )

                ra = wregs[(2 * s) % NRING]
                rb = wregs[(2 * s + 1) % NRING]
                nc.tensor.reg_load(ra, widx_sb[0:1, 2 * s: 2 * s + 1])
                nc.tensor.reg_load(rb, widx_sb[0:1, 2 * s + 1: 2 * s + 2])
                rvA = RuntimeValue(val=ra, min_val=0, max_val=WMAX)
                rvB = RuntimeValue(val=rb, min_val=0, max_val=WMAX)

                psA = pspool.tile([128, D_MODEL], f32, tag="ps")
                psB = pspool.tile([128, D_MODEL], f32, tag="ps")
                for k in range(N_KCH):
                    kk = _kchunk(k)
                    lhsT = x_sb[:kk, k * 128: k * 128 + 128]
                    nc.tensor.matmul(
                        psA[:, :], lhsT,
                        w_sb[k][:kk, bass.ds(rvA, D_MODEL)],
                        start=(k == 0), stop=(k == N_KCH - 1),
                    )
                    nc.tensor.matmul(
                        psB[:, :], lhsT,
                        w_sb[k][:kk, bass.ds(rvB, D_MODEL)],
                        start=(k == 0), stop=(k == N_KCH - 1),
                    )

                t1 = tpool.tile([128, D_MODEL], f32, tag="t")
                t2 = tpool.tile([128, D_MODEL], f32, tag="t")
                nc.scalar.mul(t1[:, :], psA[:, :], g_sb[:, 2 * s: 2 * s + 1])
                nc.scalar.mul(t2[:, :], psB[:, :], g_sb[:, 2 * s + 1: 2 * s + 2])

                o_sb = opool.tile([128, D_MODEL], bf16, tag="o")
                nc.vector.tensor_tensor(
                    o_sb[:, :], t1[:, :], t2[:, :], mybir.AluOpType.add
                )
                nc.sync.dma_start(out=y_h[s, :, :], in_=o_sb[:, :])

    nc.finalize()  # Bacc: reg graph-coloring + codegen passes, then freeze
    return nc


def _gates_np(logits, moe_masks):
    """Mirror reference _gates in numpy (fp32)."""
    lg = logits.astype(np.float32)
    m = lg.max(axis=1, keepdims=True)
    e = np.exp(lg - m)
    g = e / e.sum(axis=1, keepdims=True)
    g = g * (moe_masks == 1).astype(np.float32)
    # top-2, ties -> lower index first (matches jax.lax.top_k)
    top_idx = np.argsort(-g, axis=1, kind="stable")[:, :TOP_K]
    rows = np.arange(g.shape[0])[:, None]
    gsel = g[rows, top_idx]                                  # [B, 2]
    gsel = gsel / (gsel.sum(axis=1, keepdims=True) + EPS)
    return gsel.astype(np.float32), top_idx.astype(np.int32)


def _prep_inputs(cycle_curve_data, logits, moe_masks, W, b):
    gsel, top_idx = _gates_np(logits, moe_masks)

    x = np.ascontiguousarray(
        cycle_curve_data.reshape(B, L, FEAT).transpose(0, 2, 1)
    ).astype(np.float32, copy=False)                         # [B, 900, 128]
    xt = np.empty((B, FEAT_AUG, L), np.float32)
    xt[:, :FEAT, :] = x
    xt[:, FEAT, :] = 1.0

    w_aug = np.concatenate(
        [W.astype(np.float32), b.astype(np.float32)[:, None, :]], axis=1
    )                                                        # [E, 901, 512]
    w_host = np.zeros((N_KCH, 128, NUM_EXPERTS, D_MODEL), np.float32)
    for k in range(N_KCH):
        kk = _kchunk(k)
        w_host[k, :kk] = w_aug[:, k * 128: k * 128 + kk, :].transpose(1, 0, 2)

    in_maps = []
    for c in range(N_CORES):
        sl = slice(c * S, (c + 1) * S)
        g_rep = np.broadcast_to(
            gsel[sl].reshape(1, 2 * S), (128, 2 * S)
        ).copy()
        widx = (top_idx[sl].reshape(1, 2 * S) * D_MODEL).astype(np.int32)
        in_maps.append({
            "xt": np.ascontiguousarray(xt[sl]),
            "w": w_host,
            "g": g_rep,
            "widx": widx,
        })
    return in_maps


def kernel(cycle_curve_data, logits, moe_masks, W, b):
    if "nc" not in _CACHE:
        _CACHE["nc"] = _build_nc()
    nc = _CACHE["nc"]

    in_maps = _prep_inputs(cycle_curve_data, logits, moe_masks, W, b)

    trace = bool(int(os.environ.get("KERNEL_PROFILE", "0")))
    res = run_bass_kernel_spmd(
        nc, in_maps, core_ids=list(range(N_CORES)), trace=trace
    )
    _CACHE["last_results"] = res

    out = np.empty((B, L, D_MODEL), ml_dtypes.bfloat16)
    for c in range(N_CORES):
        out[c * S: (c + 1) * S] = res.results[c]["y"]
    return out
